# revision 13
# baseline (speedup 1.0000x reference)
"""Trainium2 Bass kernel for AttentionSequencePoolingLayer (DIN-style attention).

Reference computation (per batch b):
    att_in = concat([q, k, q-k, q*k], -1)            (T, 4E)
    h1 = relu(att_in @ W1 + b1)                      (T, 80)
    h2 = relu(h1 @ W2 + b2)                          (T, 40)
    s  = h2 @ W3 + b3                                (T, 1)
    out = (mask * s).T @ k                           (1, E)

Restructured: q is constant over T, so with W1 = [W1a; W1b; W1c; W1d]:
    h1.T = relu( (W1b-W1c).T @ kT + W1d.T @ (q (.) kT) + C[:, b] )
    C = (W1a+W1c).T @ q_b + b1   (precomputed for all batches)
The per-batch bias C is injected into the PSUM accumulation with a K=4
"selector" matmul (lhsT = 4 rows of C in natural (batch, h) layout, rhs =
0/1 block-indicator constants), so relu1 is bias-free single-instruction.

kT (keys with E on partitions) is produced by the DMA transpose XBAR from
bf16 key tiles, which are themselves loaded with gpsimd (SWDGE) casting
DMAs straight from the f32 DRAM keys -- no PE transposes, no PSUM
evacuation, no on-chip f32->bf16 conversion passes.

All matmul moving operands are bf16 (full PE rate). Weights are padded to
128 columns for fast weight load. Scores are computed on T-partitions by
per-batch mini matmuls over 128-column windows of a stride-padded h2
(tails zeroed), masked on DVE, and pooled with per-batch bf16 weight
loads. relu work is split between ACT and DVE to balance engine load.

Sharding: pure data parallel, batch dim split across 8 NeuronCores
(256 batches per core), 16-batch supergroups, 4-batch matmul groups.
"""

from contextlib import ExitStack

import numpy as np

import concourse.bass as bass
import concourse.bacc as bacc
import concourse.tile as tile
from concourse import mybir
from concourse.bass_utils import run_bass_kernel_spmd
from concourse.masks import make_identity

B, T, E = 2048, 200, 128
H1, H2 = 80, 40
N_CORES = 8
B_CORE = B // N_CORES   # 256
SG = 16                 # batches per supergroup (keys-DMA granularity)
GRP = 4                 # batches per matmul group
NG = SG // GRP          # groups per supergroup
TA = 128                # first key-row chunk (partitions)
TBR = 72                # real rows in second chunk
TBP = 80                # padded rows (multiple of 16 for the XBAR)
KB = TA + TBP           # kt columns per batch (208)
GCOL = GRP * KB         # 832
SGCOL = SG * KB         # 3328
H2W = 896               # padded h2 width (>= 3*KB + 256)
NSPL = 512              # matmul N / PSUM-bank split point

F32 = mybir.dt.float32
BF16 = mybir.dt.bfloat16
U8 = mybir.dt.uint8
AF = mybir.ActivationFunctionType
OP = mybir.AluOpType


def build(b_core=B_CORE):
    nc = bacc.Bacc("TRN2", target_bir_lowering=False, debug=False,
                   num_devices=N_CORES)
    q_d = nc.dram_tensor("query", [b_core, 1, E], F32, kind="ExternalInput")
    k_d = nc.dram_tensor("keys", [b_core, T, E], F32, kind="ExternalInput")
    m_d = nc.dram_tensor("key_masks", [b_core, 1, T], U8, kind="ExternalInput")
    w1_d = nc.dram_tensor("W1", [4 * E, H1], F32, kind="ExternalInput")
    b1_d = nc.dram_tensor("b1", [H1], F32, kind="ExternalInput")
    w2_d = nc.dram_tensor("W2", [H1, H2], F32, kind="ExternalInput")
    b2_d = nc.dram_tensor("b2", [H2], F32, kind="ExternalInput")
    w3_d = nc.dram_tensor("W3", [H2, 1], F32, kind="ExternalInput")
    b3_d = nc.dram_tensor("b3", [1], F32, kind="ExternalInput")
    out_d = nc.dram_tensor("out", [b_core, 1, E], F32, kind="ExternalOutput")

    with tile.TileContext(nc) as tc:
        _body(tc, nc, q_d, k_d, m_d, w1_d, b1_d, w2_d, b2_d, w3_d, b3_d,
              out_d, b_core)
    nc.compile()
    return nc


def _body(tc, nc, q_d, k_d, m_d, w1_d, b1_d, w2_d, b2_d, w3_d, b3_d, out_d,
          b_core):
    ctx = ExitStack()
    n_sg = b_core // SG
    with ctx:
        consts = ctx.enter_context(tc.tile_pool(name="consts", bufs=1))
        work = ctx.enter_context(tc.tile_pool(name="work", bufs=3))
        ktps_p = ctx.enter_context(
            tc.tile_pool(name="ktps_p", bufs=1, space="PSUM"))
        h1ps = ctx.enter_context(
            tc.tile_pool(name="h1ps", bufs=1, space="PSUM"))
        h2ps = ctx.enter_context(
            tc.tile_pool(name="h2ps", bufs=1, space="PSUM"))
        # shared bank: mini scores (cols 0:64) + pooling (cols 64:128) per
        # supergroup; also preamble/epilogue transpose scratch.
        bankp = ctx.enter_context(
            tc.tile_pool(name="bankp", bufs=2, space="PSUM"))

        ident = consts.tile([128, 128], F32)
        make_identity(nc, ident)
        identb = consts.tile([128, 128], BF16)
        nc.vector.tensor_copy(out=identb, in_=ident)

        # ---- weights (padded to 128 columns for fast weight load) ----
        w1s = consts.tile([128, 4, H1], F32)
        nc.sync.dma_start(out=w1s, in_=w1_d.rearrange("(f p) c -> p f c", p=128))
        w1bc = consts.tile([128, 128], BF16)
        nc.vector.memset(w1bc, 0.0)
        nc.vector.tensor_tensor(out=w1bc[:, 0:H1], in0=w1s[:, 1, :],
                                in1=w1s[:, 2, :], op=OP.subtract)
        w1db = consts.tile([128, 128], BF16)
        nc.vector.memset(w1db, 0.0)
        nc.vector.tensor_copy(out=w1db[:, 0:H1], in_=w1s[:, 3, :])
        w1ac = consts.tile([128, H1], F32)
        nc.vector.tensor_tensor(out=w1ac, in0=w1s[:, 0, :], in1=w1s[:, 2, :],
                                op=OP.add)
        w2f = consts.tile([H1, H2], F32)
        nc.sync.dma_start(out=w2f, in_=w2_d.ap())
        w2b = consts.tile([H1, 128], BF16)
        nc.vector.memset(w2b, 0.0)
        nc.vector.tensor_copy(out=w2b[:, 0:H2], in_=w2f)
        w3f = consts.tile([H2, 1], F32)
        nc.sync.dma_start(out=w3f, in_=w3_d.ap())
        w3pb = consts.tile([H2, 2], BF16)
        nc.vector.memset(w3pb, 0.0)
        nc.vector.tensor_copy(out=w3pb[:, 0:1], in_=w3f)
        b1bc = consts.tile([128, H1], F32)
        nc.sync.dma_start(
            out=b1bc, in_=bass.AP(tensor=b1_d.ap().tensor, offset=0,
                                  ap=[[0, 128], [1, H1]]))
        b2 = consts.tile([H2, 1], F32)
        nc.sync.dma_start(
            out=b2, in_=bass.AP(tensor=b2_d.ap().tensor, offset=0,
                                ap=[[1, H2], [1, 1]]))
        b3bc = consts.tile([128, 1], F32)
        nc.sync.dma_start(
            out=b3bc, in_=bass.AP(tensor=b3_d.ap().tensor, offset=0,
                                  ap=[[0, 128], [1, 1]]))

        # selector master: selm[r, q, c] = (r == q). A group whose batches
        # sit at rows 4p..4p+4 of its 32-row C chunk uses the contiguous
        # slice selm[:, 4p, 0] .. (GCOL wide). Built as a block-identity via
        # two affine selects (1.0 survives only where r - q == 0).
        selm = consts.tile([32, 32, KB], BF16)
        nc.gpsimd.memset(selm, 1.0)
        nc.gpsimd.affine_select(
            out=selm, in_=selm, compare_op=OP.is_ge, fill=0.0, base=0,
            pattern=[[-1, 32], [0, KB]], channel_multiplier=1)
        nc.gpsimd.affine_select(
            out=selm, in_=selm, compare_op=OP.is_ge, fill=0.0, base=0,
            pattern=[[1, 32], [0, KB]], channel_multiplier=-1)
        selmf = selm.rearrange("p a c -> p (a c)")

        zerob = consts.tile([128, 2 * SG], BF16)
        nc.vector.memset(zerob, 0.0)

        # ---- Q.T (E on partitions, batch on free) ----
        qt = consts.tile([128, b_core], F32)
        q_flat = q_d.rearrange("b 1 e -> b e")
        for i in range(0, b_core, 128):
            qnat = work.tile([128, E], F32, tag="qnat")
            nc.sync.dma_start(out=qnat, in_=q_flat[i:i + 128, :])
            qps = bankp.tile([128, 128], F32, tag="bank")
            nc.tensor.transpose(qps, qnat, ident)
            nc.vector.tensor_copy(out=qt[:, i:i + 128], in_=qps)

        # ---- C rows: cts[a][r, :] = (W1a+W1c).T @ q_(32a+r) + b1 ----
        cts = [consts.tile([32, 128], BF16, name=f"cts{i}")
               for i in range(b_core // 32)]
        for ct in cts:
            nc.vector.memset(ct, 0.0)
        for ci, i in enumerate(range(0, b_core, 128)):
            cps = bankp.tile([128, 128], F32, tag="bank")
            nc.tensor.matmul(cps[:, 0:H1], lhsT=qt[:, i:i + 128], rhs=w1ac,
                             start=True, stop=True)
            for a in range(4):
                nc.vector.tensor_tensor(
                    out=cts[4 * ci + a][:, 0:H1],
                    in0=cps[32 * a:32 * a + 32, 0:H1],
                    in1=b1bc[0:32, :], op=OP.add)

        # ---- masks, transposed to (t, batch); plus b3 * mask ----
        mt0 = consts.tile([TA, b_core], F32)
        mt1 = consts.tile([TBR, b_core], F32)
        m_flat = m_d.rearrange("b 1 t -> b t")
        for i in range(0, b_core, 128):
            mu8 = work.tile([128, T], U8, tag="mu8")
            nc.sync.dma_start(out=mu8, in_=m_flat[i:i + 128, :])
            mf = work.tile([128, T], F32, tag="mf")
            nc.vector.tensor_copy(out=mf, in_=mu8)
            mp0 = bankp.tile([128, 128], F32, tag="bank")
            nc.tensor.transpose(mp0, mf[:, 0:TA], ident)
            nc.vector.tensor_copy(out=mt0[:, i:i + 128], in_=mp0)
            mp1 = bankp.tile([128, 128], F32, tag="bank")
            nc.tensor.transpose(mp1[0:TBR, :], mf[:, TA:T], ident)
            nc.vector.tensor_copy(out=mt1[:, i:i + 128], in_=mp1[0:TBR, :])
        b3m0 = consts.tile([TA, b_core], F32)
        nc.vector.tensor_scalar_mul(b3m0, mt0, b3bc[0:TA, :])
        b3m1 = consts.tile([TBR, b_core], F32)
        nc.vector.tensor_scalar_mul(b3m1, mt1, b3bc[0:TBR, :])

        # pooled output, transposed: (E, batch)
        poolt = consts.tile([128, b_core], F32)

        # ---- persistent double-buffered tiles for the SG pipeline ----
        tAb2 = [consts.tile([TA, SG, E], BF16, name=f"tAb{i}")
                for i in range(2)]
        tBb2 = [consts.tile([TBP, SG, E], BF16, name=f"tBb{i}")
                for i in range(2)]
        h2p2 = [consts.tile([H2, H2W], BF16, name=f"h2pad{i}")
                for i in range(2)]
        stA2 = [consts.tile([TA, 2 * SG], BF16, name=f"stA{i}")
                for i in range(2)]
        stB2 = [consts.tile([TBP, 2 * SG], BF16, name=f"stB{i}")
                for i in range(2)]
        for t in tBb2:
            nc.vector.memset(t, 0.0)
        for t in h2p2:
            nc.vector.memset(t[:, GCOL:H2W], 0.0)
        for t in stB2:
            nc.vector.memset(t, 0.0)

        def load_sg(s):
            b0 = s * SG
            tAb, tBb = tAb2[s % 2], tBb2[s % 2]
            # casting DMAs (SWDGE): f32 DRAM -> bf16 SBUF, t on partitions
            nc.gpsimd.dma_start(
                out=tAb,
                in_=k_d[b0:b0 + SG, 0:TA, :].rearrange("b t e -> t b e"))
            nc.gpsimd.dma_start(
                out=tBb[0:TBR, :, :],
                in_=k_d[b0:b0 + SG, TA:T, :].rearrange("b t e -> t b e"))

        n_grp = b_core // GRP

        def transpose_grp(g):
            """PE-transpose group g's keys into PSUM, evacuate to bf16 kt,
            and form qk = kt * q. Returns (kt, qk) SBUF tiles."""
            s = g // NG
            tAb, tBb = tAb2[s % 2], tBb2[s % 2]
            ktp = ktps_p.tile([128, GCOL], BF16, tag="ktp")
            for j in range(GRP):
                lb = (g % NG) * GRP + j
                c = j * KB
                nc.tensor.transpose(ktp[:, c:c + TA], tAb[:, lb, :], identb)
                nc.tensor.transpose(ktp[:, c + TA:c + KB], tBb[:, lb, :],
                                    identb[0:TBP, 0:TBP])
            ktg = work.tile([128, GCOL], BF16, tag="ktg")
            nc.vector.tensor_copy(out=ktg, in_=ktp)
            qk = work.tile([128, GCOL], BF16, tag="qk")
            gb0 = g * GRP
            for j in range(GRP):
                nc.vector.tensor_scalar_mul(
                    qk[:, j * KB:(j + 1) * KB],
                    ktg[:, j * KB:(j + 1) * KB],
                    qt[:, gb0 + j:gb0 + j + 1])
            return ktg, qk

        def mlp_front(g, ktg, qk):
            gb0 = g * GRP
            h1p = h1ps.tile([128, GCOL], F32, tag="h1p")
            ct = cts[gb0 // 32]
            p0 = (gb0 % 32) * KB
            # weight-grouped: both halves per stationary operand (1 LDW each)
            nc.tensor.matmul(h1p[:, 0:NSPL], lhsT=w1bc, rhs=ktg[:, 0:NSPL],
                             start=True, stop=False)
            nc.tensor.matmul(h1p[:, NSPL:GCOL], lhsT=w1bc,
                             rhs=ktg[:, NSPL:GCOL], start=True, stop=False)
            nc.tensor.matmul(h1p[:, 0:NSPL], lhsT=w1db, rhs=qk[:, 0:NSPL],
                             start=False, stop=False)
            nc.tensor.matmul(h1p[:, NSPL:GCOL], lhsT=w1db,
                             rhs=qk[:, NSPL:GCOL], start=False, stop=False)
            nc.tensor.matmul(h1p[:, 0:NSPL], lhsT=ct,
                             rhs=selmf[:, p0:p0 + NSPL],
                             start=False, stop=True)
            nc.tensor.matmul(h1p[:, NSPL:GCOL], lhsT=ct,
                             rhs=selmf[:, p0 + NSPL:p0 + GCOL],
                             start=False, stop=True)
            h1 = work.tile([H1, GCOL], BF16, tag="h1")
            nc.scalar.activation(out=h1[:, 0:NSPL],
                                 in_=h1p[0:H1, 0:NSPL], func=AF.Relu)
            nc.vector.tensor_scalar_max(out=h1[:, NSPL:GCOL],
                                        in0=h1p[0:H1, NSPL:GCOL],
                                        scalar1=0.0)
            return h1

        def mlp_back(g, h1, bank):
            h2p = h2ps.tile([128, GCOL], F32, tag="h2p")
            h2 = h2p2[g % 2]
            nc.tensor.matmul(h2p[:, 0:NSPL], lhsT=w2b, rhs=h1[:, 0:NSPL],
                             start=True, stop=True)
            nc.tensor.matmul(h2p[:, NSPL:GCOL], lhsT=w2b,
                             rhs=h1[:, NSPL:GCOL], start=True, stop=True)
            nc.scalar.activation(out=h2[:, 0:NSPL], in_=h2p[0:H2, 0:NSPL],
                                 func=AF.Relu, bias=b2)
            nc.scalar.activation(out=h2[:, NSPL:GCOL],
                                 in_=h2p[0:H2, NSPL:GCOL],
                                 func=AF.Relu, bias=b2)

        def minis_grp(g, bank):
            h2 = h2p2[g % 2]
            for j in range(GRP):
                lb = (g % NG) * GRP + j
                c = j * KB
                nc.tensor.matmul(bank[:, 4 * lb:4 * lb + 2],
                                 lhsT=h2[:, c:c + 128], rhs=w3pb,
                                 start=True, stop=True)
                nc.tensor.matmul(bank[:, 4 * lb + 2:4 * lb + 4],
                                 lhsT=h2[:, c + 128:c + 256], rhs=w3pb,
                                 start=True, stop=True)

        def pool_sg(s, bank):
            b0 = s * SG
            tAb, tBb = tAb2[s % 2], tBb2[s % 2]
            stA, stB = stA2[s % 2], stB2[s % 2]
            minis = bank[:, 0:64].rearrange("p (b four) -> p b four", four=4)
            nc.vector.tensor_copy(out=stA, in_=zerob)
            stAv = stA.rearrange("p (b two) -> p b two", two=2)[:, :, 0]
            nc.vector.tensor_tensor(out=stAv, in0=minis[:, :, 0],
                                    in1=mt0[:, b0:b0 + SG], op=OP.mult)
            nc.vector.tensor_tensor(out=stAv, in0=stAv,
                                    in1=b3m0[:, b0:b0 + SG], op=OP.add)
            nc.vector.tensor_copy(out=stB[0:TBR, :], in_=zerob[0:TBR, :])
            stBv = stB.rearrange("p (b two) -> p b two", two=2)[:, :, 0]
            nc.vector.tensor_tensor(out=stBv[0:TBR, :],
                                    in0=minis[0:TBR, :, 2],
                                    in1=mt1[:, b0:b0 + SG], op=OP.mult)
            nc.vector.tensor_tensor(out=stBv[0:TBR, :], in0=stBv[0:TBR, :],
                                    in1=b3m1[:, b0:b0 + SG], op=OP.add)
            for lb in range(SG):
                c = 64 + 4 * lb
                nc.tensor.matmul(bank[:, c:c + 2], lhsT=tAb[:, lb, :],
                                 rhs=stA[:, 2 * lb:2 * lb + 2],
                                 start=True, stop=True)
                nc.tensor.matmul(bank[:, c + 2:c + 4], lhsT=tBb[:, lb, :],
                                 rhs=stB[:, 2 * lb:2 * lb + 2],
                                 start=True, stop=True)
            poolv = bank[:, 64:128].rearrange("p (b four) -> p b four", four=4)
            pltmp = work.tile([128, SG], F32, tag="pltmp")
            nc.vector.tensor_copy(out=pltmp, in_=poolv[:, :, 0])
            nc.vector.tensor_tensor(out=poolt[:, b0:b0 + SG],
                                    in0=poolv[:, :, 2], in1=pltmp, op=OP.add)

        # ---- HAM warm-up: ~6us of back-to-back matmuls so the PE clock
        # gate opens (4/8 -> 8/8) before the main pipeline starts ----
        prime = h1ps.tile([128, GCOL], F32, tag="h1p")
        for _ in range(40):
            nc.tensor.matmul(prime[:, 0:128], lhsT=identb,
                             rhs=identb, start=True, stop=True)

        # ---- software-pipelined group loop ----
        # PE order per iteration: L1(g) -> T(g+1) -> minis(g-1) [-> pool] ->
        # L2(g), so the PE always has independent work while ACT/DVE run
        # relu1(g) and prepare kt/qk for g+1.
        load_sg(0)
        cur = transpose_grp(0)
        banks = {}
        h1_prev = None
        for g in range(n_grp):
            s = g // NG
            if g >= 1 and (g - 1) % NG == NG - 1:
                # SG boundary: drain minis+pool of s-1 BEFORE load_sg(s+1)
                # overwrites the tAb/tBb buffers pool still reads.
                minis_grp(g - 1, banks[(g - 1) // NG])
                pool_sg((g - 1) // NG, banks.pop((g - 1) // NG))
            if g % NG == 0:
                if s + 1 < n_sg:
                    load_sg(s + 1)
                banks[s] = bankp.tile([128, 128], F32, tag="bank",
                                      name="bank")
            h1 = mlp_front(g, cur[0], cur[1])
            cur = transpose_grp(g + 1) if g + 1 < n_grp else None
            if g >= 1 and (g - 1) % NG != NG - 1:
                minis_grp(g - 1, banks[(g - 1) // NG])
            mlp_back(g, h1, banks[s])
        minis_grp(n_grp - 1, banks[n_sg - 1])
        pool_sg(n_sg - 1, banks.pop(n_sg - 1))

        # ---- final: transpose pooled back to (batch, E) and store ----
        out_flat = out_d.rearrange("b 1 e -> b e")
        for i in range(0, b_core, 128):
            ops = bankp.tile([128, 128], F32, tag="bank")
            nc.tensor.transpose(ops, poolt[:, i:i + 128], ident)
            onat = work.tile([128, E], F32, tag="onat")
            nc.vector.tensor_copy(out=onat, in_=ops)
            nc.sync.dma_start(out=out_flat[i:i + 128, :], in_=onat)


_NC_CACHE = {}


def _get_nc(b_core=B_CORE):
    if b_core not in _NC_CACHE:
        _NC_CACHE[b_core] = build(b_core)
    return _NC_CACHE[b_core]


def kernel(query, keys, key_masks, W1, b1, W2, b2, W3, b3, _trace=False):
    query = np.ascontiguousarray(query, dtype=np.float32)
    keys = np.ascontiguousarray(keys, dtype=np.float32)
    masks_u8 = np.ascontiguousarray(key_masks).view(np.uint8)
    nc = _get_nc()
    in_maps = []
    for c in range(N_CORES):
        sl = slice(c * B_CORE, (c + 1) * B_CORE)
        in_maps.append({
            "query": query[sl],
            "keys": keys[sl],
            "key_masks": masks_u8[sl],
            "W1": np.asarray(W1, dtype=np.float32),
            "b1": np.asarray(b1, dtype=np.float32),
            "W2": np.asarray(W2, dtype=np.float32),
            "b2": np.asarray(b2, dtype=np.float32),
            "W3": np.asarray(W3, dtype=np.float32),
            "b3": np.asarray(b3, dtype=np.float32),
        })
    res = run_bass_kernel_spmd(nc, in_maps, list(range(N_CORES)), trace=_trace)
    out = np.concatenate([res.results[c]["out"] for c in range(N_CORES)], axis=0)
    if _trace:
        kernel.last_exec_time_ns = res.exec_time_ns
        kernel.last_results = res
    return out.astype(np.float32)


kernel.last_exec_time_ns = None
kernel.last_results = None


# revision 14
# speedup vs baseline: 1.4650x; 1.4650x over previous
"""Trainium2 Bass kernel for AttentionSequencePoolingLayer (DIN-style attention).

Reference computation (per batch b):
    att_in = concat([q, k, q-k, q*k], -1)            (T, 4E)
    h1 = relu(att_in @ W1 + b1)                      (T, 80)
    h2 = relu(h1 @ W2 + b2)                          (T, 40)
    s  = h2 @ W3 + b3                                (T, 1)
    out = (mask * s).T @ k                           (1, E)

Algebraic restructuring (cuts layer-1 FLOPs 4x): q is constant over T, so
with W1 = [W1a; W1b; W1c; W1d] (blocks of E rows):
    h1.T = relu( (W1b-W1c).T @ k.T  +  W1d.T @ (q (.) k.T)  +  C[:, b] )
    C = (W1a+W1c).T @ Q.T + b1      (one matmul for all batches)
The per-batch bias C[:, b] is injected into the same PSUM accumulation with a
K=2 "selector" matmul (lhsT = two C columns, rhs = 0/1 selector rows), so the
relu evacuation needs no per-batch bias and is a single wide op.

Scores are produced directly on T-partitions by small transposed matmuls
(lhsT = h2.T column slices, rhs = padded W3), masked during evacuation with
pre-transposed masks (b3 rides in as b3*mask), then pooling accumulates
poolT[:, b] = keys_nat.T @ masked_scores as single-column matmuls (bf16
weights -> fast weight load). fp32r everywhere else (N >= 256 keeps full PE
rate; N=1 is ISA-illegal for fp32r so score/pool columns are padded to N=2).

Sharding: pure data parallel, batch dim split across 8 NeuronCores
(256 batches per core), 16-batch supergroups (two ~1MB strided DMAs each),
2-batch matmul groups (moving dim N=400).
"""

from contextlib import ExitStack

import numpy as np

import concourse.bass as bass
import concourse.bacc as bacc
import concourse.tile as tile
from concourse import mybir
from concourse.bass_utils import run_bass_kernel_spmd
from concourse.masks import make_identity

B, T, E = 2048, 200, 128
H1, H2 = 80, 40
N_CORES = 8
B_CORE = B // N_CORES   # 256
SG = 16                 # batches per supergroup (keys-DMA granularity)
GRP = 2                 # batches per matmul group (N = GRP*T = 400)
TA, TB = 128, T - 128   # key-row split across partitions

F32 = mybir.dt.float32
F32R = mybir.dt.float32r
BF16 = mybir.dt.bfloat16
U8 = mybir.dt.uint8
AF = mybir.ActivationFunctionType
OP = mybir.AluOpType


def build(b_core=B_CORE):
    nc = bacc.Bacc("TRN2", target_bir_lowering=False, debug=False,
                   num_devices=N_CORES)
    q_d = nc.dram_tensor("query", [b_core, 1, E], F32, kind="ExternalInput")
    k_d = nc.dram_tensor("keys", [b_core, T, E], F32R, kind="ExternalInput")
    m_d = nc.dram_tensor("key_masks", [b_core, 1, T], U8, kind="ExternalInput")
    w1_d = nc.dram_tensor("W1", [4 * E, H1], F32, kind="ExternalInput")
    b1_d = nc.dram_tensor("b1", [H1], F32, kind="ExternalInput")
    w2_d = nc.dram_tensor("W2", [H1, H2], F32, kind="ExternalInput")
    b2_d = nc.dram_tensor("b2", [H2], F32, kind="ExternalInput")
    w3_d = nc.dram_tensor("W3", [H2, 1], F32, kind="ExternalInput")
    b3_d = nc.dram_tensor("b3", [1], F32, kind="ExternalInput")
    out_d = nc.dram_tensor("out", [b_core, 1, E], F32, kind="ExternalOutput")

    with tile.TileContext(nc) as tc:
        _body(tc, nc, q_d, k_d, m_d, w1_d, b1_d, w2_d, b2_d, w3_d, b3_d,
              out_d, b_core)
    nc.compile()
    return nc


def _body(tc, nc, q_d, k_d, m_d, w1_d, b1_d, w2_d, b2_d, w3_d, b3_d, out_d,
          b_core):
    ctx = ExitStack()
    n_g = b_core // GRP
    with ctx:
        consts = ctx.enter_context(tc.tile_pool(name="consts", bufs=1))
        prep = ctx.enter_context(tc.tile_pool(name="prep", bufs=2))
        # shared psum bank: preamble/epilogue scratch + supergroup columns
        pp_ps = ctx.enter_context(
            tc.tile_pool(name="pp_ps", bufs=1, space="PSUM"))

        ident = consts.tile([128, 128], F32)
        make_identity(nc, ident)
        ident_r = consts.tile([128, 128], F32R)
        nc.vector.tensor_copy(out=ident_r, in_=ident)

        # ---- weights ----
        w1s = consts.tile([128, 4, H1], F32)
        nc.sync.dma_start(out=w1s, in_=w1_d.rearrange("(f p) c -> p f c", p=128))
        w1bc = consts.tile([128, H1], F32R)
        nc.vector.tensor_tensor(out=w1bc, in0=w1s[:, 1, :], in1=w1s[:, 2, :],
                                op=OP.subtract)
        w1ac = consts.tile([128, H1], F32R)
        nc.vector.tensor_tensor(out=w1ac, in0=w1s[:, 0, :], in1=w1s[:, 2, :],
                                op=OP.add)
        w1db = consts.tile([128, H1], BF16)
        nc.vector.tensor_copy(out=w1db, in_=w1s[:, 3, :])
        w2f = consts.tile([H1, H2], F32)
        nc.sync.dma_start(out=w2f, in_=w2_d.ap())
        w2 = consts.tile([H1, H2], F32R)
        nc.vector.tensor_copy(out=w2, in_=w2f)
        w3f = consts.tile([H2, 1], F32)
        nc.sync.dma_start(out=w3f, in_=w3_d.ap())
        w3pf = consts.tile([H2, 2], F32)
        nc.vector.memset(w3pf, 0.0)
        nc.vector.tensor_copy(out=w3pf[:, 0:1], in_=w3f)
        w3pb = consts.tile([H2, 2], BF16)
        nc.vector.tensor_copy(out=w3pb, in_=w3pf)
        b1 = consts.tile([H1, 1], F32)
        nc.sync.dma_start(
            out=b1, in_=bass.AP(tensor=b1_d.ap().tensor, offset=0,
                                ap=[[1, H1], [1, 1]]))
        b2 = consts.tile([H2, 1], F32)
        nc.sync.dma_start(
            out=b2, in_=bass.AP(tensor=b2_d.ap().tensor, offset=0,
                                ap=[[1, H2], [1, 1]]))
        b3bc = consts.tile([128, 1], F32)
        nc.sync.dma_start(
            out=b3bc, in_=bass.AP(tensor=b3_d.ap().tensor, offset=0,
                                  ap=[[0, 128], [1, 1]]))
        zerob = consts.tile([128, 2 * SG], BF16)
        nc.vector.memset(zerob, 0.0)

        # ---- Q.T (E on partitions, batch on free) ----
        qt = consts.tile([128, b_core], F32R)
        q_flat = q_d.rearrange("b 1 e -> b e")
        for i in range(0, b_core, 128):
            cnt = min(128, b_core - i)
            qnat = prep.tile([128, E], F32, tag="qnat")
            nc.sync.dma_start(out=qnat[:cnt, :], in_=q_flat[i:i + cnt, :])
            qps = pp_ps.tile([128, 256], F32, tag="pps")
            nc.tensor.transpose(qps[:, :cnt], qnat[:cnt, :], ident[:cnt, :cnt])
            nc.vector.tensor_copy(out=qt[:, i:i + cnt], in_=qps[:, :cnt])

        # ---- C = (W1a+W1c).T @ Q.T + b1, repacked for K=2 selector matmuls:
        # ct_all[j, g, :] = C[:, GRP*g + j]
        cps = pp_ps.tile([128, 256], F32, tag="pps")
        nc.tensor.matmul(cps[:H1, :b_core], lhsT=w1ac, rhs=qt,
                         start=True, stop=True)
        csb = consts.tile([H1, b_core], F32)
        nc.scalar.activation(out=csb, in_=cps[:H1, :b_core], func=AF.Identity,
                             bias=b1)

        # ---- masks, transposed to (t, batch); plus b3 * mask ----
        mt0 = consts.tile([TA, b_core], F32)
        mt1 = consts.tile([TB, b_core], F32)
        m_flat = m_d.rearrange("b 1 t -> b t")
        for i in range(0, b_core, 128):
            cnt = min(128, b_core - i)
            mu8 = prep.tile([128, T], U8, tag="mu8")
            nc.sync.dma_start(out=mu8[:cnt, :], in_=m_flat[i:i + cnt, :])
            mf = prep.tile([128, T], F32, tag="mf")
            nc.vector.tensor_copy(out=mf[:cnt, :], in_=mu8[:cnt, :])
            mp0 = pp_ps.tile([128, 256], F32, tag="pps")
            nc.tensor.transpose(mp0[:TA, :cnt], mf[:cnt, 0:TA],
                                ident[:cnt, :cnt])
            nc.vector.tensor_copy(out=mt0[:, i:i + cnt], in_=mp0[:TA, :cnt])
            mp1 = pp_ps.tile([128, 256], F32, tag="pps")
            nc.tensor.transpose(mp1[:TB, :cnt], mf[:cnt, TA:T],
                                ident[:cnt, :cnt])
            nc.vector.tensor_copy(out=mt1[:, i:i + cnt], in_=mp1[:TB, :cnt])
        b3m0 = consts.tile([TA, b_core], F32)
        nc.vector.tensor_scalar_mul(b3m0, mt0, b3bc[0:TA, :])
        b3m1 = consts.tile([TB, b_core], F32)
        nc.vector.tensor_scalar_mul(b3m1, mt1, b3bc[0:TB, :])

        # pooled output, transposed: (E, batch)
        poolt_sb = consts.tile([128, b_core], F32)

        # ---- main pipeline pools ----
        kstA = ctx.enter_context(tc.tile_pool(name="kstA", bufs=2))
        kstB = ctx.enter_context(tc.tile_pool(name="kstB", bufs=2))
        ktp = ctx.enter_context(tc.tile_pool(name="ktp", bufs=6))
        work = ctx.enter_context(tc.tile_pool(name="work", bufs=6))
        stp = ctx.enter_context(tc.tile_pool(name="stp", bufs=2))
        pk_ps = ctx.enter_context(tc.tile_pool(name="pk_ps", bufs=3, space="PSUM"))
        h1_ps = ctx.enter_context(tc.tile_pool(name="h1_ps", bufs=2, space="PSUM"))
        h2_ps = ctx.enter_context(tc.tile_pool(name="h2_ps", bufs=1, space="PSUM"))
        sm_ps = ctx.enter_context(tc.tile_pool(name="sm_ps", bufs=1, space="PSUM"))

        n_sg = (b_core + SG - 1) // SG
        NCOL = GRP * T  # 400

        for sg in range(n_sg):
            b0 = sg * SG
            nb = min(SG, b_core - b0)
            # big strided loads: natural keys, t on partitions, batch on free
            tA = kstA.tile([TA, SG, E], F32R, tag="tA")
            nc.sync.dma_start(
                out=tA[:, :nb, :],
                in_=k_d[b0:b0 + nb, 0:TA, :].rearrange("b t e -> t b e"))
            tB = kstB.tile([TB, SG, E], F32R, tag="tB")
            nc.sync.dma_start(
                out=tB[:, :nb, :],
                in_=k_d[b0:b0 + nb, TA:T, :].rearrange("b t e -> t b e"))
            # bf16 copies for the pooling weight loads (fast weight load)
            tAb = kstA.tile([TA, SG, E], BF16, tag="tAb")
            nc.vector.tensor_copy(out=tAb[:, :nb, :],
                                  in_=tA[:, :nb, :].bitcast(F32))
            tBb = kstB.tile([TB, SG, E], BF16, tag="tBb")
            nc.vector.tensor_copy(out=tBb[:, :nb, :],
                                  in_=tB[:, :nb, :].bitcast(F32))

            # per-supergroup psum bank: score columns and pooled columns share
            # one bank; every matmul into it is atomic (start+stop) over
            # disjoint columns, so bank-wide has_written clears are harmless.
            smbig = sm_ps.tile([128, 8 * SG], F32, tag="smbig")
            stA_ps = smbig[:, 0:2 * SG]
            stB_ps = smbig[0:TB, 2 * SG:4 * SG]
            plTA_ps = smbig[:, 4 * SG:6 * SG]
            plTB_ps = smbig[:, 6 * SG:8 * SG]

            # groups are emitted pairwise, phase by phase, so each
            # cross-engine handoff has a full phase of slack to complete
            # before the consumer issues on its engine. The score minis of
            # the previous pair are emitted between this pair's L1 chain and
            # L2 so the PE has work while relu1 runs on ACT.
            def emit_minis(h2_list):
                for lb, h2 in h2_list:
                    for j in range(GRP):
                        c = j * T
                        o = 2 * (lb + j)
                        nc.tensor.matmul(stA_ps[:, o:o + 2],
                                         lhsT=h2[:, c:c + TA], rhs=w3pb,
                                         start=True, stop=True)
                        nc.tensor.matmul(stB_ps[:, o:o + 2],
                                         lhsT=h2[:, c + TA:c + T], rhs=w3pb,
                                         start=True, stop=True)

            pending = []
            for g0 in range(0, nb // GRP, 2):
                pair = [g for g in (g0, g0 + 1) if g < nb // GRP]
                st = {}
                for g in pair:
                    lb = GRP * g
                    ktps = pk_ps.tile([128, NCOL], F32R, tag="ktps")
                    for j in range(GRP):
                        c = j * T
                        nc.tensor.transpose(ktps[:, c:c + TA],
                                            tA[:, lb + j, :], ident_r)
                        nc.tensor.transpose(ktps[:, c + TA:c + T],
                                            tB[:, lb + j, :],
                                            ident_r[:TB, :TB])
                    st[g] = {"ktps": ktps}
                for g in pair:
                    kt = ktp.tile([128, NCOL], F32R, tag="kt")
                    nc.vector.tensor_copy(out=kt, in_=st[g]["ktps"])
                    st[g]["kt"] = kt
                for g in pair:
                    gb = b0 + GRP * g
                    kt = st[g]["kt"]
                    qk = ktp.tile([128, NCOL], BF16, tag="qk")
                    for j in range(GRP):
                        nc.vector.tensor_scalar_mul(
                            qk[:, j * T:(j + 1) * T],
                            kt[:, j * T:(j + 1) * T].bitcast(F32),
                            qt[:, gb + j:gb + j + 1].bitcast(F32))
                    st[g]["qk"] = qk
                for g in pair:
                    gb = b0 + GRP * g
                    h1p = h1_ps.tile([H1, NCOL], F32, tag="h1p")
                    nc.tensor.matmul(h1p, lhsT=w1bc, rhs=st[g]["kt"],
                                     start=True, stop=False)
                    nc.tensor.matmul(h1p, lhsT=w1db, rhs=st[g]["qk"],
                                     start=False, stop=True)
                    st[g]["h1p"] = h1p
                for g in pair:
                    gb = b0 + GRP * g
                    h1 = work.tile([H1, NCOL], F32R, tag="h1")
                    for j in range(GRP):
                        nc.scalar.activation(
                            out=h1[:, j * T:(j + 1) * T],
                            in_=st[g]["h1p"][:, j * T:(j + 1) * T],
                            func=AF.Relu, bias=csb[:, gb + j:gb + j + 1])
                    st[g]["h1"] = h1
                emit_minis(pending)
                pending = []
                for g in pair:
                    h2p = h2_ps.tile([H2, NCOL], F32, tag="h2p")
                    nc.tensor.matmul(h2p, lhsT=w2, rhs=st[g]["h1"],
                                     start=True, stop=True)
                    st[g]["h2p"] = h2p
                for g in pair:
                    h2 = work.tile([H2, NCOL], BF16, tag="h2")
                    nc.scalar.activation(out=h2, in_=st[g]["h2p"],
                                         func=AF.Relu, bias=b2)
                    st[g]["h2"] = h2
                    pending.append((GRP * g, h2))
            emit_minis(pending)
            pending = []

            # masked scores: sT_m = sT * m + b3 * m   (b3m precomputed)
            stA_s = stA_ps.rearrange("p (b two) -> p b two", two=2)[:, :, 0]
            stB_s = stB_ps.rearrange("p (b two) -> p b two", two=2)[:, :, 0]
            stA = stp.tile([TA, 2 * SG], BF16, tag="stA")
            nc.vector.tensor_copy(out=stA, in_=zerob[:TA, :])
            stAv = stA.rearrange("p (b two) -> p b two", two=2)[:, :, 0]
            nc.vector.tensor_tensor(out=stAv[:, :nb], in0=stA_s[:, :nb],
                                    in1=mt0[:, b0:b0 + nb], op=OP.mult)
            nc.vector.tensor_tensor(out=stAv[:, :nb], in0=stAv[:, :nb],
                                    in1=b3m0[:, b0:b0 + nb], op=OP.add)
            stB = stp.tile([TB, 2 * SG], BF16, tag="stB")
            nc.vector.tensor_copy(out=stB, in_=zerob[:TB, :])
            stBv = stB.rearrange("p (b two) -> p b two", two=2)[:, :, 0]
            nc.vector.tensor_tensor(out=stBv[:, :nb], in0=stB_s[:, :nb],
                                    in1=mt1[:, b0:b0 + nb], op=OP.mult)
            nc.vector.tensor_tensor(out=stBv[:, :nb], in0=stBv[:, :nb],
                                    in1=b3m1[:, b0:b0 + nb], op=OP.add)

            # pooling: poolT[:, b] = knat_A.T @ sTm_A + knat_B.T @ sTm_B
            # (halves land in separate psum columns, summed on evacuation)
            for lb in range(nb):
                nc.tensor.matmul(plTA_ps[:, 2 * lb:2 * lb + 2],
                                 lhsT=tAb[:, lb, :],
                                 rhs=stA[:, 2 * lb:2 * lb + 2],
                                 start=True, stop=True)
                nc.tensor.matmul(plTB_ps[:, 2 * lb:2 * lb + 2],
                                 lhsT=tBb[:, lb, :],
                                 rhs=stB[:, 2 * lb:2 * lb + 2],
                                 start=True, stop=True)
            plA_s = plTA_ps.rearrange("p (b two) -> p b two", two=2)[:, :, 0]
            plB_s = plTB_ps.rearrange("p (b two) -> p b two", two=2)[:, :, 0]
            pltmp = stp.tile([128, SG], F32, tag="pltmp")
            nc.vector.tensor_copy(out=pltmp[:, :nb], in_=plA_s[:, :nb])
            nc.vector.tensor_tensor(out=poolt_sb[:, b0:b0 + nb],
                                    in0=plB_s[:, :nb], in1=pltmp[:, :nb],
                                    op=OP.add)

        # ---- final: transpose pooled back to (batch, E) and store ----
        out_flat = out_d.rearrange("b 1 e -> b e")
        for i in range(0, b_core, 128):
            cnt = min(128, b_core - i)
            ops = pp_ps.tile([128, 256], F32, tag="pps")
            nc.tensor.transpose(ops[:cnt, :128], poolt_sb[:, i:i + cnt], ident)
            onat = prep.tile([128, E], F32, tag="onat")
            nc.vector.tensor_copy(out=onat[:cnt, :], in_=ops[:cnt, :128])
            nc.sync.dma_start(out=out_flat[i:i + cnt, :], in_=onat[:cnt, :])


_NC_CACHE = {}


def _get_nc(b_core=B_CORE):
    if b_core not in _NC_CACHE:
        _NC_CACHE[b_core] = build(b_core)
    return _NC_CACHE[b_core]


def kernel(query, keys, key_masks, W1, b1, W2, b2, W3, b3, _trace=False):
    query = np.ascontiguousarray(query, dtype=np.float32)
    keys = np.ascontiguousarray(keys, dtype=np.float32)
    masks_u8 = np.ascontiguousarray(key_masks).view(np.uint8)
    nc = _get_nc()
    in_maps = []
    for c in range(N_CORES):
        sl = slice(c * B_CORE, (c + 1) * B_CORE)
        in_maps.append({
            "query": query[sl],
            "keys": keys[sl],
            "key_masks": masks_u8[sl],
            "W1": np.asarray(W1, dtype=np.float32),
            "b1": np.asarray(b1, dtype=np.float32),
            "W2": np.asarray(W2, dtype=np.float32),
            "b2": np.asarray(b2, dtype=np.float32),
            "W3": np.asarray(W3, dtype=np.float32),
            "b3": np.asarray(b3, dtype=np.float32),
        })
    res = run_bass_kernel_spmd(nc, in_maps, list(range(N_CORES)), trace=_trace)
    out = np.concatenate([res.results[c]["out"] for c in range(N_CORES)], axis=0)
    if _trace:
        kernel.last_exec_time_ns = res.exec_time_ns
        kernel.last_results = res
    return out.astype(np.float32)


kernel.last_exec_time_ns = None
kernel.last_results = None



# revision 17
# speedup vs baseline: 1.5903x; 1.0856x over previous
"""Trainium2 Bass kernel for AttentionSequencePoolingLayer (DIN-style attention).

Reference computation (per batch b):
    att_in = concat([q, k, q-k, q*k], -1)            (T, 4E)
    h1 = relu(att_in @ W1 + b1)                      (T, 80)
    h2 = relu(h1 @ W2 + b2)                          (T, 40)
    s  = h2 @ W3 + b3                                (T, 1)
    out = (mask * s).T @ k                           (1, E)

Algebraic restructuring (cuts layer-1 FLOPs 4x): q is constant over T, so
with W1 = [W1a; W1b; W1c; W1d] (blocks of E rows):
    h1.T = relu( (W1b-W1c).T @ k.T  +  W1d.T @ (q (.) k.T)  +  C[:, b] )
    C = (W1a+W1c).T @ Q.T + b1      (one matmul for all batches)
The per-batch bias C[:, b] is injected into the same PSUM accumulation with a
K=2 "selector" matmul (lhsT = two C columns, rhs = 0/1 selector rows), so the
relu evacuation needs no per-batch bias and is a single wide op.

Scores are produced directly on T-partitions by small transposed matmuls
(lhsT = h2.T column slices, rhs = padded W3), masked during evacuation with
pre-transposed masks (b3 rides in as b3*mask), then pooling accumulates
poolT[:, b] = keys_nat.T @ masked_scores as single-column matmuls (bf16
weights -> fast weight load). fp32r everywhere else (N >= 256 keeps full PE
rate; N=1 is ISA-illegal for fp32r so score/pool columns are padded to N=2).

Sharding: pure data parallel, batch dim split across 8 NeuronCores
(256 batches per core), 16-batch supergroups (two ~1MB strided DMAs each),
2-batch matmul groups (moving dim N=400).
"""

from contextlib import ExitStack

import numpy as np

import concourse.bass as bass
import concourse.bacc as bacc
import concourse.tile as tile
from concourse import mybir
from concourse.bass_utils import run_bass_kernel_spmd
from concourse.masks import make_identity

B, T, E = 2048, 200, 128
H1, H2 = 80, 40
N_CORES = 8
B_CORE = B // N_CORES   # 256
SG = 16                 # batches per supergroup (keys-DMA granularity)
GRP = 2                 # batches per matmul group (N = GRP*T = 400)
TA, TB = 128, T - 128   # key-row split across partitions

F32 = mybir.dt.float32
F32R = mybir.dt.float32r
BF16 = mybir.dt.bfloat16
U8 = mybir.dt.uint8
AF = mybir.ActivationFunctionType
OP = mybir.AluOpType


def build(b_core=B_CORE):
    nc = bacc.Bacc("TRN2", target_bir_lowering=False, debug=False,
                   num_devices=N_CORES)
    q_d = nc.dram_tensor("query", [b_core, 1, E], F32, kind="ExternalInput")
    k_d = nc.dram_tensor("keys", [b_core, T, E], F32R, kind="ExternalInput")
    m_d = nc.dram_tensor("key_masks", [b_core, 1, T], U8, kind="ExternalInput")
    w1_d = nc.dram_tensor("W1", [4 * E, H1], F32, kind="ExternalInput")
    b1_d = nc.dram_tensor("b1", [H1], F32, kind="ExternalInput")
    w2_d = nc.dram_tensor("W2", [H1, H2], F32, kind="ExternalInput")
    b2_d = nc.dram_tensor("b2", [H2], F32, kind="ExternalInput")
    w3_d = nc.dram_tensor("W3", [H2, 1], F32, kind="ExternalInput")
    b3_d = nc.dram_tensor("b3", [1], F32, kind="ExternalInput")
    out_d = nc.dram_tensor("out", [b_core, 1, E], F32, kind="ExternalOutput")

    with tile.TileContext(nc) as tc:
        _body(tc, nc, q_d, k_d, m_d, w1_d, b1_d, w2_d, b2_d, w3_d, b3_d,
              out_d, b_core)
    nc.compile()
    return nc


def _body(tc, nc, q_d, k_d, m_d, w1_d, b1_d, w2_d, b2_d, w3_d, b3_d, out_d,
          b_core):
    ctx = ExitStack()
    n_g = b_core // GRP
    with ctx:
        consts = ctx.enter_context(tc.tile_pool(name="consts", bufs=1))
        prep = ctx.enter_context(tc.tile_pool(name="prep", bufs=2))
        # shared psum bank: preamble/epilogue scratch + supergroup columns
        pp_ps = ctx.enter_context(
            tc.tile_pool(name="pp_ps", bufs=1, space="PSUM"))

        ident = consts.tile([128, 128], F32)
        make_identity(nc, ident)
        ident_r = consts.tile([128, 128], F32R)
        nc.vector.tensor_copy(out=ident_r, in_=ident)
        identb = consts.tile([128, 128], BF16)
        nc.vector.tensor_copy(out=identb, in_=ident)

        # ---- weights ----
        w1s = consts.tile([128, 4, H1], F32)
        nc.sync.dma_start(out=w1s, in_=w1_d.rearrange("(f p) c -> p f c", p=128))
        w1bc = consts.tile([128, H1], BF16)
        nc.vector.tensor_tensor(out=w1bc, in0=w1s[:, 1, :], in1=w1s[:, 2, :],
                                op=OP.subtract)
        w1ac = consts.tile([128, H1], F32R)
        nc.vector.tensor_tensor(out=w1ac, in0=w1s[:, 0, :], in1=w1s[:, 2, :],
                                op=OP.add)
        w1db = consts.tile([128, H1], BF16)
        nc.vector.tensor_copy(out=w1db, in_=w1s[:, 3, :])
        w2f = consts.tile([H1, H2], F32)
        nc.sync.dma_start(out=w2f, in_=w2_d.ap())
        w2 = consts.tile([H1, H2], F32R)
        nc.vector.tensor_copy(out=w2, in_=w2f)
        w3f = consts.tile([H2, 1], F32)
        nc.sync.dma_start(out=w3f, in_=w3_d.ap())
        w3pf = consts.tile([H2, 2], F32)
        nc.vector.memset(w3pf, 0.0)
        nc.vector.tensor_copy(out=w3pf[:, 0:1], in_=w3f)
        w3pb = consts.tile([H2, 2], BF16)
        nc.vector.tensor_copy(out=w3pb, in_=w3pf)
        b1 = consts.tile([H1, 1], F32)
        nc.sync.dma_start(
            out=b1, in_=bass.AP(tensor=b1_d.ap().tensor, offset=0,
                                ap=[[1, H1], [1, 1]]))
        b2 = consts.tile([H2, 1], F32)
        nc.sync.dma_start(
            out=b2, in_=bass.AP(tensor=b2_d.ap().tensor, offset=0,
                                ap=[[1, H2], [1, 1]]))
        b3bc = consts.tile([128, 1], F32)
        nc.sync.dma_start(
            out=b3bc, in_=bass.AP(tensor=b3_d.ap().tensor, offset=0,
                                  ap=[[0, 128], [1, 1]]))
        zerob = consts.tile([128, 2 * SG], BF16)
        nc.vector.memset(zerob, 0.0)

        # ---- Q.T (E on partitions, batch on free) ----
        qt = consts.tile([128, b_core], F32R)
        q_flat = q_d.rearrange("b 1 e -> b e")
        for i in range(0, b_core, 128):
            cnt = min(128, b_core - i)
            qnat = prep.tile([128, E], F32, tag="qnat")
            nc.sync.dma_start(out=qnat[:cnt, :], in_=q_flat[i:i + cnt, :])
            qps = pp_ps.tile([128, 256], F32, tag="pps")
            nc.tensor.transpose(qps[:, :cnt], qnat[:cnt, :], ident[:cnt, :cnt])
            nc.vector.tensor_copy(out=qt[:, i:i + cnt], in_=qps[:, :cnt])

        # ---- C = (W1a+W1c).T @ Q.T + b1, repacked for K=2 selector matmuls:
        # ct_all[j, g, :] = C[:, GRP*g + j]
        cps = pp_ps.tile([128, 256], F32, tag="pps")
        nc.tensor.matmul(cps[:H1, :b_core], lhsT=w1ac, rhs=qt,
                         start=True, stop=True)
        csb = consts.tile([H1, b_core], F32)
        nc.scalar.activation(out=csb, in_=cps[:H1, :b_core], func=AF.Identity,
                             bias=b1)

        # ---- masks, transposed to (t, batch); plus b3 * mask ----
        mt0 = consts.tile([TA, b_core], F32)
        mt1 = consts.tile([TB, b_core], F32)
        m_flat = m_d.rearrange("b 1 t -> b t")
        for i in range(0, b_core, 128):
            cnt = min(128, b_core - i)
            mu8 = prep.tile([128, T], U8, tag="mu8")
            nc.sync.dma_start(out=mu8[:cnt, :], in_=m_flat[i:i + cnt, :])
            mf = prep.tile([128, T], F32, tag="mf")
            nc.vector.tensor_copy(out=mf[:cnt, :], in_=mu8[:cnt, :])
            mp0 = pp_ps.tile([128, 256], F32, tag="pps")
            nc.tensor.transpose(mp0[:TA, :cnt], mf[:cnt, 0:TA],
                                ident[:cnt, :cnt])
            nc.vector.tensor_copy(out=mt0[:, i:i + cnt], in_=mp0[:TA, :cnt])
            mp1 = pp_ps.tile([128, 256], F32, tag="pps")
            nc.tensor.transpose(mp1[:TB, :cnt], mf[:cnt, TA:T],
                                ident[:cnt, :cnt])
            nc.vector.tensor_copy(out=mt1[:, i:i + cnt], in_=mp1[:TB, :cnt])
        b3m0 = consts.tile([TA, b_core], F32)
        nc.vector.tensor_scalar_mul(b3m0, mt0, b3bc[0:TA, :])
        b3m1 = consts.tile([TB, b_core], F32)
        nc.vector.tensor_scalar_mul(b3m1, mt1, b3bc[0:TB, :])

        # pooled output, transposed: (E, batch)
        poolt_sb = consts.tile([128, b_core], F32)

        # persistent h2 ring: [40, 464] with zeroed tail so both score minis
        # use 128-column (FWL-eligible) weight windows
        h2ring = [consts.tile([H2, 464], BF16, name=f"h2r{i}")
                  for i in range(4)]
        for t in h2ring:
            nc.vector.memset(t[:, 400:464], 0.0)

        # ---- main pipeline pools ----
        kstA = ctx.enter_context(tc.tile_pool(name="kstA", bufs=2))
        kstB = ctx.enter_context(tc.tile_pool(name="kstB", bufs=2))
        ktp = ctx.enter_context(tc.tile_pool(name="ktp", bufs=6))
        work = ctx.enter_context(tc.tile_pool(name="work", bufs=6))
        stp = ctx.enter_context(tc.tile_pool(name="stp", bufs=2))
        pk_ps = ctx.enter_context(tc.tile_pool(name="pk_ps", bufs=3, space="PSUM"))
        h1_ps = ctx.enter_context(tc.tile_pool(name="h1_ps", bufs=2, space="PSUM"))
        h2_ps = ctx.enter_context(tc.tile_pool(name="h2_ps", bufs=1, space="PSUM"))
        sm_ps = ctx.enter_context(tc.tile_pool(name="sm_ps", bufs=1, space="PSUM"))

        n_sg = (b_core + SG - 1) // SG
        NCOL = GRP * T  # 400

        for sg in range(n_sg):
            b0 = sg * SG
            nb = min(SG, b_core - b0)
            # big strided loads: natural keys, t on partitions, batch on free
            tA = kstA.tile([TA, SG, E], F32R, tag="tA")
            nc.sync.dma_start(
                out=tA[:, :nb, :],
                in_=k_d[b0:b0 + nb, 0:TA, :].rearrange("b t e -> t b e"))
            tB = kstB.tile([TB, SG, E], F32R, tag="tB")
            nc.sync.dma_start(
                out=tB[:, :nb, :],
                in_=k_d[b0:b0 + nb, TA:T, :].rearrange("b t e -> t b e"))
            # bf16 copies (pooling weights + transpose inputs), emitted
            # per-pair below so the first pair isn't gated on the full SG
            tAb = kstA.tile([TA, SG, E], BF16, tag="tAb")
            tBb = kstB.tile([TB, SG, E], BF16, tag="tBb")

            def conv(p):
                lb0 = 2 * GRP * p
                cnt = min(2 * GRP, nb - lb0)
                if cnt <= 0:
                    return
                nc.vector.tensor_copy(out=tAb[:, lb0:lb0 + cnt, :],
                                      in_=tA[:, lb0:lb0 + cnt, :].bitcast(F32))
                nc.vector.tensor_copy(out=tBb[:, lb0:lb0 + cnt, :],
                                      in_=tB[:, lb0:lb0 + cnt, :].bitcast(F32))

            # per-supergroup psum bank: score columns and pooled columns share
            # one bank; every matmul into it is atomic (start+stop) over
            # disjoint columns, so bank-wide has_written clears are harmless.
            smbig = sm_ps.tile([128, 8 * SG], F32, tag="smbig")
            stA_ps = smbig[:, 0:2 * SG]
            stB_ps = smbig[:, 2 * SG:4 * SG]
            plTA_ps = smbig[:, 4 * SG:6 * SG]
            plTB_ps = smbig[:, 6 * SG:8 * SG]

            # groups are emitted pairwise, phase by phase, so each
            # cross-engine handoff has a full phase of slack to complete
            # before the consumer issues on its engine. The score minis of
            # the previous pair are emitted between this pair's L1 chain and
            # L2 so the PE has work while relu1 runs on ACT.
            def emit_minis(h2_list):
                for lb, h2 in h2_list:
                    for j in range(GRP):
                        c = j * T
                        o = 2 * (lb + j)
                        nc.tensor.matmul(stA_ps[:, o:o + 2],
                                         lhsT=h2[:, c:c + TA], rhs=w3pb,
                                         start=True, stop=True)
                        nc.tensor.matmul(stB_ps[:, o:o + 2],
                                         lhsT=h2[:, c + 128:c + 256],
                                         rhs=w3pb, start=True, stop=True)

            pending = []
            conv(0)
            for p, g0 in enumerate(range(0, nb // GRP, 2)):
                pair = [g for g in (g0, g0 + 1) if g < nb // GRP]
                st = {}
                for g in pair:
                    lb = GRP * g
                    ktps = pk_ps.tile([128, NCOL], BF16, tag="ktps")
                    for j in range(GRP):
                        c = j * T
                        nc.tensor.transpose(ktps[:, c:c + TA],
                                            tAb[:, lb + j, :], identb)
                        nc.tensor.transpose(ktps[:, c + TA:c + T],
                                            tBb[:, lb + j, :],
                                            identb[:TB, :TB])
                    st[g] = {"ktps": ktps}
                conv(p + 1)
                for g in pair:
                    kt = ktp.tile([128, NCOL], BF16, tag="kt")
                    nc.vector.tensor_copy(out=kt, in_=st[g]["ktps"])
                    st[g]["kt"] = kt
                for g in pair:
                    gb = b0 + GRP * g
                    kt = st[g]["kt"]
                    qk = ktp.tile([128, NCOL], BF16, tag="qk")
                    for j in range(GRP):
                        nc.vector.tensor_scalar_mul(
                            qk[:, j * T:(j + 1) * T],
                            kt[:, j * T:(j + 1) * T],
                            qt[:, gb + j:gb + j + 1].bitcast(F32))
                    st[g]["qk"] = qk
                for g in pair:
                    st[g]["h1p"] = h1_ps.tile([H1, NCOL], F32, tag="h1p",
                                              name="h1p")
                for g in pair:
                    nc.tensor.matmul(st[g]["h1p"], lhsT=w1bc, rhs=st[g]["kt"],
                                     start=True, stop=False)
                for g in pair:
                    nc.tensor.matmul(st[g]["h1p"], lhsT=w1db, rhs=st[g]["qk"],
                                     start=False, stop=True)
                for g in pair:
                    gb = b0 + GRP * g
                    h1 = work.tile([H1, NCOL], F32R, tag="h1")
                    for j in range(GRP):
                        nc.scalar.activation(
                            out=h1[:, j * T:(j + 1) * T],
                            in_=st[g]["h1p"][:, j * T:(j + 1) * T],
                            func=AF.Relu, bias=csb[:, gb + j:gb + j + 1])
                    st[g]["h1"] = h1
                emit_minis(pending)
                pending = []
                for g in pair:
                    h2p = h2_ps.tile([H2, NCOL], F32, tag="h2p")
                    nc.tensor.matmul(h2p, lhsT=w2, rhs=st[g]["h1"],
                                     start=True, stop=True)
                    st[g]["h2p"] = h2p
                for g in pair:
                    h2 = h2ring[(sg * (n_g // n_sg) + g) % 4]
                    nc.scalar.activation(out=h2[:, 0:NCOL], in_=st[g]["h2p"],
                                         func=AF.Relu, bias=b2)
                    st[g]["h2"] = h2
                    pending.append((GRP * g, h2))
            emit_minis(pending)
            pending = []

            # masked scores: sT_m = sT * m + b3 * m   (b3m precomputed)
            stA_s = stA_ps.rearrange("p (b two) -> p b two", two=2)[:, :, 0]
            stB_s = stB_ps.rearrange("p (b two) -> p b two", two=2)[:, :, 0]
            stA = stp.tile([TA, 2 * SG], BF16, tag="stA")
            nc.vector.tensor_copy(out=stA, in_=zerob[:TA, :])
            stAv = stA.rearrange("p (b two) -> p b two", two=2)[:, :, 0]
            nc.vector.tensor_tensor(out=stAv[:, :nb], in0=stA_s[:, :nb],
                                    in1=mt0[:, b0:b0 + nb], op=OP.mult)
            nc.vector.tensor_tensor(out=stAv[:, :nb], in0=stAv[:, :nb],
                                    in1=b3m0[:, b0:b0 + nb], op=OP.add)
            stB = stp.tile([TB, 2 * SG], BF16, tag="stB")
            nc.vector.tensor_copy(out=stB, in_=zerob[:TB, :])
            stBv = stB.rearrange("p (b two) -> p b two", two=2)[:, :, 0]
            nc.vector.tensor_tensor(out=stBv[:, :nb],
                                    in0=stB_s[0:TB, :nb],
                                    in1=mt1[:, b0:b0 + nb], op=OP.mult)
            nc.vector.tensor_tensor(out=stBv[:, :nb], in0=stBv[:, :nb],
                                    in1=b3m1[:, b0:b0 + nb], op=OP.add)

            # pooling: poolT[:, b] = knat_A.T @ sTm_A + knat_B.T @ sTm_B
            # (halves land in separate psum columns, summed on evacuation)
            for lb in range(nb):
                nc.tensor.matmul(plTA_ps[:, 2 * lb:2 * lb + 2],
                                 lhsT=tAb[:, lb, :],
                                 rhs=stA[:, 2 * lb:2 * lb + 2],
                                 start=True, stop=True)
                nc.tensor.matmul(plTB_ps[:, 2 * lb:2 * lb + 2],
                                 lhsT=tBb[:, lb, :],
                                 rhs=stB[:, 2 * lb:2 * lb + 2],
                                 start=True, stop=True)
            plA_s = plTA_ps.rearrange("p (b two) -> p b two", two=2)[:, :, 0]
            plB_s = plTB_ps.rearrange("p (b two) -> p b two", two=2)[:, :, 0]
            pltmp = stp.tile([128, SG], F32, tag="pltmp")
            nc.vector.tensor_copy(out=pltmp[:, :nb], in_=plA_s[:, :nb])
            nc.vector.tensor_tensor(out=poolt_sb[:, b0:b0 + nb],
                                    in0=plB_s[:, :nb], in1=pltmp[:, :nb],
                                    op=OP.add)

        # ---- final: transpose pooled back to (batch, E) and store ----
        out_flat = out_d.rearrange("b 1 e -> b e")
        for i in range(0, b_core, 128):
            cnt = min(128, b_core - i)
            ops = pp_ps.tile([128, 256], F32, tag="pps")
            nc.tensor.transpose(ops[:cnt, :128], poolt_sb[:, i:i + cnt], ident)
            onat = prep.tile([128, E], F32, tag="onat")
            nc.vector.tensor_copy(out=onat[:cnt, :], in_=ops[:cnt, :128])
            nc.sync.dma_start(out=out_flat[i:i + cnt, :], in_=onat[:cnt, :])


_NC_CACHE = {}


def _get_nc(b_core=B_CORE):
    if b_core not in _NC_CACHE:
        _NC_CACHE[b_core] = build(b_core)
    return _NC_CACHE[b_core]


def kernel(query, keys, key_masks, W1, b1, W2, b2, W3, b3, _trace=False):
    query = np.ascontiguousarray(query, dtype=np.float32)
    keys = np.ascontiguousarray(keys, dtype=np.float32)
    masks_u8 = np.ascontiguousarray(key_masks).view(np.uint8)
    nc = _get_nc()
    in_maps = []
    for c in range(N_CORES):
        sl = slice(c * B_CORE, (c + 1) * B_CORE)
        in_maps.append({
            "query": query[sl],
            "keys": keys[sl],
            "key_masks": masks_u8[sl],
            "W1": np.asarray(W1, dtype=np.float32),
            "b1": np.asarray(b1, dtype=np.float32),
            "W2": np.asarray(W2, dtype=np.float32),
            "b2": np.asarray(b2, dtype=np.float32),
            "W3": np.asarray(W3, dtype=np.float32),
            "b3": np.asarray(b3, dtype=np.float32),
        })
    res = run_bass_kernel_spmd(nc, in_maps, list(range(N_CORES)), trace=_trace)
    out = np.concatenate([res.results[c]["out"] for c in range(N_CORES)], axis=0)
    if _trace:
        kernel.last_exec_time_ns = res.exec_time_ns
        kernel.last_results = res
    return out.astype(np.float32)


kernel.last_exec_time_ns = None
kernel.last_results = None

